# revision 2
# baseline (speedup 1.0000x reference)
"""Modulated conv2d (StyleGAN-2 style, B=16 C=128 HxW=128x128 K=3) on 8 TRN2
NeuronCores, data-parallel over batch (2 samples/core).

v1 design (vs the 150us baseline):
  * The style linear / modulation / demodulation are folded into the weights
    on the host (0.004% of the FLOPs); the device runs the grouped conv with
    per-sample premodulated fp16 weights.  This removes the style-param DMA +
    PE style matmul + DVE wmod/sq + ACT sqrt + DVE recip chain that gated conv
    start at ~15.5us in the baseline.
  * fp16 everywhere (x, weights, output): fp16 matmuls run at bf16 speed with
    more mantissa (rel err ~1e-4), and the fp16 output halves the output DMA
    (host upcasts to f32 after gather).
  * ~6 warmup matmuls on garbage data right after the preamble pull the PE
    HAM clock-gate to 2.4GHz while the first DMAs are still in flight.
  * Evictions (PSUM -> SBUF staging, plain copy now that demod is folded)
    alternate between ACT and DVE; output DMAs are issued from the Sync
    engine, input weight DMAs from ACT.
  * Conv: per 4-row output block, 9 tap matmuls (K=C_in, M=C_out, N=512)
    accumulate fp32 in PSUM; tap shift is a strided 3D rhs view into the
    130x130 zero-padded image - no im2col.  6 rotating PSUM banks, 12 output
    staging buffers.

Raw Bass with manual semaphores: this toolchain's walrus accepts only ONE
sync-wait command per instruction, so every engine-pair dependency (including
same-engine RAW) is guarded by an explicit single-wait `wait_ge`.
"""

import sys

sys.path.insert(0, "/opt/trn_rl_repo")

import numpy as np

import concourse.bass as bass
from concourse import mybir
from concourse.bass_utils import run_bass_kernel_spmd

B, C, H, W, KS = 16, 128, 128, 128, 3
NCORES = 8
SPC = B // NCORES          # samples per core = 2
HP = H + 2                 # padded height/width = 130
NT = KS * KS               # 9 taps

R = 4                      # output rows per conv block (N = R*W = 512)
NPS = 6                    # rotating conv PSUM banks
NOB = 12                   # output staging buffers
NB = H // R                # conv blocks per sample = 32
NWARM = 6                  # PE warmup matmuls (HAM clock-gate)
CHUNK_BNDS = [0, 6, 48, 90, 130]   # x DMA chunk row boundaries (padded rows)


def _chunk_of_block(b):
    """First x chunk that covers padded rows needed by output block b."""
    need = R * b + R + 1
    for c in range(len(CHUNK_BNDS) - 1):
        if need < CHUNK_BNDS[c + 1]:
            return c
    raise AssertionError


F32 = mybir.dt.float32
F16 = mybir.dt.float16
MULT = mybir.AluOpType.mult


def build_program():
    nc = bass.Bass(trn_type="TRN2", target_bir_lowering=False, debug=False)

    xpad_d = nc.dram_tensor("xpad", [SPC, C, HP, HP], F16, kind="ExternalInput").ap()
    wp_d = nc.dram_tensor("wp", [C, SPC * NT * C], F16, kind="ExternalInput").ap()
    y_d = nc.dram_tensor("y", [SPC, C, H, W], F16, kind="ExternalOutput").ap()

    xs = nc.alloc_sbuf_tensor("xs", [C, SPC, HP, HP], F16).ap()
    wms = nc.alloc_sbuf_tensor("wms", [C, SPC, NT * C], F16).ap()
    outsb = nc.alloc_sbuf_tensor("outsb", [C, NOB, R * W], F16).ap()

    cps = [nc.alloc_psum_tensor(f"cps{j}", [C, R * W], F32).ap() for j in range(NPS)]
    warm = nc.alloc_psum_tensor("warm", [C, R * W], F32).ap()

    sem_x = [nc.alloc_semaphore(f"sx{i}") for i in range(SPC * 4)]
    sem_w = [nc.alloc_semaphore(f"sw{s}") for s in range(SPC)]
    sem_pe_blk = nc.alloc_semaphore("pe_blk")
    sem_ev = [nc.alloc_semaphore("ev_a"), nc.alloc_semaphore("ev_d")]
    sem_od = [nc.alloc_semaphore(f"sod{j}") for j in range(NOB)]

    with nc.Block() as blk:

        @blk.sync
        def _(eng):
            # input image chunks; weight DMAs ride on ACT's queue
            def xchunk(s, ci):
                r0, r1 = CHUNK_BNDS[ci], CHUNK_BNDS[ci + 1]
                eng.dma_start(
                    out=xs[:, s : s + 1, r0:r1, :],
                    in_=xpad_d[s : s + 1, :, r0:r1, :],
                ).then_inc(sem_x[4 * s + ci], 16)

            for s in range(SPC):
                for ci in range(4):
                    xchunk(s, ci)

            # output DMAs (evictions alternate ACT/DVE; wait the right sem)
            for gb in range(SPC * NB):
                s, b = gb // NB, gb % NB
                eng.wait_ge(sem_ev[gb % 2], gb // 2 + 1)
                eng.dma_start(
                    out=y_d[s : s + 1, :, R * b : R * b + R, :],
                    in_=outsb[:, gb % NOB : gb % NOB + 1, :],
                ).then_inc(sem_od[gb % NOB], 16)

        @blk.scalar
        def _(eng):
            # per-sample premodulated weights: first DMA gates conv start
            for s in range(SPC):
                eng.dma_start(
                    out=wms[:, s : s + 1, :],
                    in_=wp_d[:, s * NT * C : (s + 1) * NT * C],
                ).then_inc(sem_w[s], 16)
            # even-block evictions: plain PSUM -> SBUF fp16 copy
            for gb in range(0, SPC * NB, 2):
                eng.wait_ge(sem_pe_blk, gb + 1)
                if gb >= NOB:
                    eng.wait_ge(sem_od[gb % NOB], 16 * (gb // NOB))
                eng.copy(outsb[:, gb % NOB : gb % NOB + 1, :],
                         cps[gb % NPS]).then_inc(sem_ev[0], 1)

        @blk.vector
        def _(eng):
            # odd-block evictions
            for gb in range(1, SPC * NB, 2):
                eng.wait_ge(sem_pe_blk, gb + 1)
                if gb >= NOB:
                    eng.wait_ge(sem_od[gb % NOB], 16 * (gb // NOB))
                eng.tensor_scalar(outsb[:, gb % NOB : gb % NOB + 1, :],
                                  cps[gb % NPS], 1.0, None,
                                  MULT).then_inc(sem_ev[1], 1)

        @blk.tensor
        def _(eng):
            # warmup: pull the HAM clock gate to 2.4GHz on garbage data while
            # the first DMAs are in flight (results discarded)
            for _ in range(NWARM):
                eng.matmul(
                    out=warm,
                    lhsT=wms[:, 0:1, 0:C],
                    rhs=xs[:, 1:2, 0:R, 0:W],
                    start=True,
                    stop=True,
                )

            def conv_block(s, b, gb):
                if b == 0 or _chunk_of_block(b) != _chunk_of_block(b - 1):
                    eng.wait_ge(sem_x[4 * s + _chunk_of_block(b)], 16)
                if gb >= NPS:
                    prev = gb - NPS
                    eng.wait_ge(sem_ev[prev % 2], prev // 2 + 1)
                for kh in range(KS):
                    for kw in range(KS):
                        t = kh * KS + kw
                        inst = eng.matmul(
                            out=cps[gb % NPS],
                            lhsT=wms[:, s : s + 1, t * C : (t + 1) * C],
                            rhs=xs[:, s : s + 1, R * b + kh : R * b + kh + R,
                                   kw : kw + W],
                            start=(t == 0),
                            stop=(t == NT - 1),
                        )
                inst.then_inc(sem_pe_blk, 1)

            for s in range(SPC):
                eng.wait_ge(sem_w[s], 16)
                for b in range(NB):
                    conv_block(s, b, s * NB + b)

    return nc


def _host_prep(x, w, weight, mod_w, mod_b):
    f = np.float32
    x = np.asarray(x, f)
    w = np.asarray(w, f)
    weight = np.asarray(weight, f)
    mod_w = np.asarray(mod_w, f)
    mod_b = np.asarray(mod_b, f)

    # style modulation + demodulation folded into per-sample weights
    s = (w @ mod_w.T + mod_b) + 1.0                      # [B, C]
    wgt = weight[None] * s[:, None, :, None, None]       # [B, O, I, K, K]
    d = 1.0 / np.sqrt((wgt * wgt).sum(axis=(2, 3, 4)) + 1e-8)   # [B, O]
    wmod = wgt * d[:, :, None, None, None]

    # wp[i, s*9C + t*C + o] = wmod[sample, o, i, kh, kw],  t = kh*3 + kw
    wT = np.ascontiguousarray(wmod.transpose(2, 0, 3, 4, 1)).reshape(C, B, NT * C)

    xpad = np.zeros((B, C, HP, HP), np.float16)
    xpad[:, :, 1 : H + 1, 1 : W + 1] = x.astype(np.float16)

    in_maps = []
    for core in range(NCORES):
        s0 = SPC * core
        in_maps.append({
            "xpad": np.ascontiguousarray(xpad[s0 : s0 + SPC]),
            "wp": np.ascontiguousarray(
                wT[:, s0 : s0 + SPC].reshape(C, SPC * NT * C)).astype(np.float16),
        })
    return in_maps


def _gather(res):
    return np.concatenate(
        [res.results[i]["y"].astype(np.float32) for i in range(NCORES)], axis=0)


_cached = {}


def kernel(x, w, weight, mod_w, mod_b):
    if "nc" not in _cached:
        _cached["nc"] = build_program()
    nc = _cached["nc"]
    in_maps = _host_prep(x, w, weight, mod_w, mod_b)
    res = run_bass_kernel_spmd(nc, in_maps, list(range(NCORES)))
    return _gather(res)


if __name__ == "__main__":
    from concourse.bass_utils import compile_bass_kernel
    import tempfile

    nc = build_program()
    d = tempfile.mkdtemp()
    neff = compile_bass_kernel(nc, d)
    print("compiled OK:", neff)


# revision 5
# speedup vs baseline: 1.5392x; 1.5392x over previous
"""Modulated conv2d (StyleGAN-2 style, B=16 C=128 HxW=128x128 K=3) on 8 TRN2
NeuronCores, data-parallel over batch (2 samples/core).

v1 design (vs the 150us baseline):
  * The style linear / modulation / demodulation are folded into the weights
    on the host (0.004% of the FLOPs); the device runs the grouped conv with
    per-sample premodulated fp16 weights.  This removes the style-param DMA +
    PE style matmul + DVE wmod/sq + ACT sqrt + DVE recip chain that gated conv
    start at ~15.5us in the baseline.
  * fp16 everywhere (x, weights, output): fp16 matmuls run at bf16 speed with
    more mantissa (rel err ~1e-4), and the fp16 output halves the output DMA
    (host upcasts to f32 after gather).
  * ~6 warmup matmuls on garbage data right after the preamble pull the PE
    HAM clock-gate to 2.4GHz while the first DMAs are still in flight.
  * Evictions (PSUM -> SBUF staging, plain copy now that demod is folded)
    alternate between ACT and DVE; output DMAs are issued from the Sync
    engine, input weight DMAs from ACT.
  * Conv: per 4-row output block, 9 tap matmuls (K=C_in, M=C_out, N=512)
    accumulate fp32 in PSUM; tap shift is a strided 3D rhs view into the
    130x130 zero-padded image - no im2col.  6 rotating PSUM banks, 12 output
    staging buffers.

Raw Bass with manual semaphores: this toolchain's walrus accepts only ONE
sync-wait command per instruction, so every engine-pair dependency (including
same-engine RAW) is guarded by an explicit single-wait `wait_ge`.
"""

import sys

sys.path.insert(0, "/opt/trn_rl_repo")

import numpy as np

import concourse.bass as bass
from concourse import mybir
from concourse.bass_utils import run_bass_kernel_spmd

B, C, H, W, KS = 16, 128, 128, 128, 3
NCORES = 8
SPC = B // NCORES          # samples per core = 2
HP = H + 2                 # padded height/width = 130
NT = KS * KS               # 9 taps

R = 4                      # output rows per conv block (N = R*W = 512)
NPS = 6                    # rotating conv PSUM banks
NOB = 12                   # output staging buffers
NB = H // R                # conv blocks per sample = 32
NWARM = 6                  # PE warmup matmuls (HAM clock-gate)
CHUNK_BNDS = [0, 6, 48, 90, 130]   # x DMA chunk row boundaries (padded rows)


def _chunk_of_block(b):
    """First x chunk that covers padded rows needed by output block b."""
    need = R * b + R + 1
    for c in range(len(CHUNK_BNDS) - 1):
        if need < CHUNK_BNDS[c + 1]:
            return c
    raise AssertionError


F32 = mybir.dt.float32
F16 = mybir.dt.float16
BF16 = mybir.dt.bfloat16
MULT = mybir.AluOpType.mult


def build_program():
    nc = bass.Bass(trn_type="TRN2", target_bir_lowering=False, debug=False)

    xpad_d = nc.dram_tensor("xpad", [SPC, C, HP, HP], BF16, kind="ExternalInput").ap()
    wp_d = nc.dram_tensor("wp", [C, SPC * NT * C], BF16, kind="ExternalInput").ap()
    y_d = nc.dram_tensor("y", [SPC, C, H, W], F16, kind="ExternalOutput").ap()

    xs = nc.alloc_sbuf_tensor("xs", [C, SPC, HP, HP], BF16).ap()
    wms = nc.alloc_sbuf_tensor("wms", [C, SPC, NT * C], BF16).ap()
    outsb = nc.alloc_sbuf_tensor("outsb", [C, NOB, R * W], F16).ap()

    cps = [nc.alloc_psum_tensor(f"cps{j}", [C, R * W], F32).ap() for j in range(NPS)]
    warm = nc.alloc_psum_tensor("warm", [C, R * W], F32).ap()

    sem_x = [nc.alloc_semaphore(f"sx{i}") for i in range(SPC * 4)]
    sem_w = [nc.alloc_semaphore(f"sw{s}") for s in range(SPC)]
    sem_pe_blk = nc.alloc_semaphore("pe_blk")
    sem_ev = [nc.alloc_semaphore("ev_a"), nc.alloc_semaphore("ev_d")]
    sem_od = [nc.alloc_semaphore(f"sod{j}") for j in range(NOB)]

    with nc.Block() as blk:

        @blk.sync
        def _(eng):
            # input image chunks; weight DMAs ride on ACT's queue
            def xchunk(s, ci):
                r0, r1 = CHUNK_BNDS[ci], CHUNK_BNDS[ci + 1]
                eng.dma_start(
                    out=xs[:, s : s + 1, r0:r1, :],
                    in_=xpad_d[s : s + 1, :, r0:r1, :],
                ).then_inc(sem_x[4 * s + ci], 16)

            # stagger the big x chunks so the conv-gating weight DMA (on
            # ACT's queue) isn't starved at the shared DMA engines
            xchunk(0, 0)
            xchunk(0, 1)
            eng.wait_ge(sem_w[0], 16)
            xchunk(0, 2)
            xchunk(0, 3)
            xchunk(1, 0)
            eng.wait_ge(sem_pe_blk, 8)
            for ci in range(1, 4):
                xchunk(1, ci)

            # output DMAs (evictions alternate ACT/DVE; wait the right sem)
            for gb in range(SPC * NB):
                s, b = gb // NB, gb % NB
                eng.wait_ge(sem_ev[gb % 2], gb // 2 + 1)
                eng.dma_start(
                    out=y_d[s : s + 1, :, R * b : R * b + R, :],
                    in_=outsb[:, gb % NOB : gb % NOB + 1, :],
                ).then_inc(sem_od[gb % NOB], 16)

        @blk.scalar
        def _(eng):
            # per-sample premodulated weights: first DMA gates conv start
            for s in range(SPC):
                eng.dma_start(
                    out=wms[:, s : s + 1, :],
                    in_=wp_d[:, s * NT * C : (s + 1) * NT * C],
                ).then_inc(sem_w[s], 16)
            # even-block evictions: plain PSUM -> SBUF fp16 copy
            for gb in range(0, SPC * NB, 2):
                eng.wait_ge(sem_pe_blk, gb + 1)
                if gb >= NOB:
                    eng.wait_ge(sem_od[gb % NOB], 16 * (gb // NOB))
                eng.copy(outsb[:, gb % NOB : gb % NOB + 1, :],
                         cps[gb % NPS]).then_inc(sem_ev[0], 1)

        @blk.vector
        def _(eng):
            # odd-block evictions
            for gb in range(1, SPC * NB, 2):
                eng.wait_ge(sem_pe_blk, gb + 1)
                if gb >= NOB:
                    eng.wait_ge(sem_od[gb % NOB], 16 * (gb // NOB))
                eng.tensor_scalar(outsb[:, gb % NOB : gb % NOB + 1, :],
                                  cps[gb % NPS], 1.0, None,
                                  MULT).then_inc(sem_ev[1], 1)

        @blk.tensor
        def _(eng):
            # warmup: pull the HAM clock gate to 2.4GHz on garbage data while
            # the first DMAs are in flight (results discarded)
            for _ in range(NWARM):
                eng.matmul(
                    out=warm,
                    lhsT=wms[:, 0:1, 0:C],
                    rhs=xs[:, 1:2, 0:R, 0:W],
                    start=True,
                    stop=True,
                )

            def conv_block(s, b, gb):
                if b == 0 or _chunk_of_block(b) != _chunk_of_block(b - 1):
                    eng.wait_ge(sem_x[4 * s + _chunk_of_block(b)], 16)
                if gb >= NPS:
                    prev = gb - NPS
                    eng.wait_ge(sem_ev[prev % 2], prev // 2 + 1)
                for kh in range(KS):
                    for kw in range(KS):
                        t = kh * KS + kw
                        inst = eng.matmul(
                            out=cps[gb % NPS],
                            lhsT=wms[:, s : s + 1, t * C : (t + 1) * C],
                            rhs=xs[:, s : s + 1, R * b + kh : R * b + kh + R,
                                   kw : kw + W],
                            start=(t == 0),
                            stop=(t == NT - 1),
                        )
                inst.then_inc(sem_pe_blk, 1)

            for s in range(SPC):
                eng.wait_ge(sem_w[s], 16)
                for b in range(NB):
                    conv_block(s, b, s * NB + b)

    return nc


def _host_prep(x, w, weight, mod_w, mod_b):
    f = np.float32
    x = np.asarray(x, f)
    w = np.asarray(w, f)
    weight = np.asarray(weight, f)
    mod_w = np.asarray(mod_w, f)
    mod_b = np.asarray(mod_b, f)

    # style modulation + demodulation folded into per-sample weights
    s = (w @ mod_w.T + mod_b) + 1.0                      # [B, C]
    wgt = weight[None] * s[:, None, :, None, None]       # [B, O, I, K, K]
    d = 1.0 / np.sqrt((wgt * wgt).sum(axis=(2, 3, 4)) + 1e-8)   # [B, O]
    wmod = wgt * d[:, :, None, None, None]

    import ml_dtypes
    bf16 = ml_dtypes.bfloat16

    # wp[i, s*9C + t*C + o] = wmod[sample, o, i, kh, kw],  t = kh*3 + kw
    wT = np.ascontiguousarray(wmod.transpose(2, 0, 3, 4, 1)).reshape(C, B, NT * C)

    xpad = np.zeros((B, C, HP, HP), bf16)
    xpad[:, :, 1 : H + 1, 1 : W + 1] = x.astype(bf16)

    in_maps = []
    for core in range(NCORES):
        s0 = SPC * core
        in_maps.append({
            "xpad": np.ascontiguousarray(xpad[s0 : s0 + SPC]),
            "wp": np.ascontiguousarray(
                wT[:, s0 : s0 + SPC].reshape(C, SPC * NT * C)).astype(bf16),
        })
    return in_maps


def _gather(res):
    return np.concatenate(
        [res.results[i]["y"].astype(np.float32) for i in range(NCORES)], axis=0)


_cached = {}


def kernel(x, w, weight, mod_w, mod_b):
    if "nc" not in _cached:
        _cached["nc"] = build_program()
    nc = _cached["nc"]
    in_maps = _host_prep(x, w, weight, mod_w, mod_b)
    res = run_bass_kernel_spmd(nc, in_maps, list(range(NCORES)))
    return _gather(res)


if __name__ == "__main__":
    from concourse.bass_utils import compile_bass_kernel
    import tempfile

    nc = build_program()
    d = tempfile.mkdtemp()
    neff = compile_bass_kernel(nc, d)
    print("compiled OK:", neff)


# revision 7
# speedup vs baseline: 1.5905x; 1.0334x over previous
"""Modulated conv2d via 1D Winograd F(2,3) along H, on 8 TRN2 NeuronCores,
data-parallel over batch (2 samples/core).

The 3x3 conv is decomposed as 3 column-taps of a 1D 3-tap conv along H, and
that 1D conv runs in the Winograd F(2,3) domain: per output row-pair i the
host precomputes the 4-point input transform

    X0 = x[2i-1]-x[2i+1]   X1 = x[2i]+x[2i+1]
    X2 = x[2i+1]-x[2i]     X3 = x[2i]-x[2i+2]      (padded, bf16)

and the modulated+demodulated+G-transformed weights g~[p,kw] (style linear,
modulation, demod all folded on host - 0.01% of the FLOPs).  The device runs
12 matmuls (4 Winograd points x 3 column taps, K=C_in, M=C_out, N=256) per
4-output-row unit instead of direct conv's 18 - a 1.5x PE reduction - then
reconstructs  y_even = M0+M1+M2,  y_odd = M1-M2-M3  post-PSUM:

    ACT : evict M0..M3 -> SBUF fp16 (one [C,1024] copy per unit) - the ONLY
          PSUM reader, so a unit's PSUM region recycles after a 1-hop chain
    DVE : per unit-pair, all-SBUF fp16 2x ops: u = M1+M2, v = M1-M2,
          y_even = u+M0, y_odd = v-M3 -> fp16 staging (even/odd rows)
    Sync: output DMA per pair (fp16; host upcasts after gather)

PSUM rotates through FOUR [C,1024] unit regions (evict chain ~1.3us vs a
~4us budget).  X~ ships tile-major so input chunks are contiguous 2D DMAs;
all input rides ACT's HWDGE ring FIFO in consumption order (weights first,
long-tail chunks interleaved into the evict loop, chunk sizes matched to
the measured contended supply rate so no chunk-end gates the conv), while
the Sync ring carries only output - no head-of-line blocking.  ~8 warmup
matmuls pull the PE HAM clock-gate to 2.4GHz during the DMA ramp.

Raw Bass with manual semaphores (single-wait walrus): every cross- and
same-engine dependency (incl. DVE same-engine RAW) is an explicit wait_ge.
"""

import sys

sys.path.insert(0, "/opt/trn_rl_repo")

import numpy as np

import concourse.bass as bass
from concourse import mybir
from concourse.bass_utils import run_bass_kernel_spmd

B, C, H, W, KS = 16, 128, 128, 128, 3
NCORES = 8
SPC = B // NCORES          # samples per core = 2
WP = W + 2                 # padded width = 130
NTL = H // 2               # 64 row-pair tiles per sample
P4 = 4                     # Winograd points
TPU = 2                    # tiles per unit (4 output rows)
NU = NTL // TPU            # 32 units per sample
NPAIR = SPC * NU // 2      # 32 unit-pairs total
NMS = 4                    # Ms staging slots (units)
NOBP = 6                   # output staging slots (pairs, 8 rows each)
NWARM = 8
# x-tile DMA chunk boundaries, supply-matched to conv consumption (~0.41us
# per tile supplied vs 0.66us consumed; chunks gate at their END so they
# must grow geometrically, b <= 1.6a + 2)
TCHS = [[0, 2, 8, 16, 27, 42, 64], [0, 20, 38, 64]]
NCH = [len(t) - 1 for t in TCHS]


def _chunk_of_unit(s, u):
    need = TPU * u + TPU - 1
    for c in range(NCH[s]):
        if need < TCHS[s][c + 1]:
            return c
    raise AssertionError


F32 = mybir.dt.float32
F16 = mybir.dt.float16
BF16 = mybir.dt.bfloat16
ADD = mybir.AluOpType.add
SUB = mybir.AluOpType.subtract


def build_program():
    nc = bass.Bass(trn_type="TRN2", target_bir_lowering=False, debug=False)

    # tile-major input transform: [s, c, tile, p, col]
    xt_d = nc.dram_tensor("xt", [SPC, C, NTL, P4, WP], BF16,
                          kind="ExternalInput").ap()
    wq_d = nc.dram_tensor("wq", [C, SPC * 12 * C], BF16,
                          kind="ExternalInput").ap()
    y_d = nc.dram_tensor("y", [SPC, C, H, W], F16, kind="ExternalOutput").ap()

    xs = nc.alloc_sbuf_tensor("xs", [C, SPC, NTL, P4, WP], BF16).ap()
    wqs = nc.alloc_sbuf_tensor("wqs", [C, SPC, 12 * C], BF16).ap()
    ms = nc.alloc_sbuf_tensor("ms", [C, NMS, P4 * 256], F16).ap()
    us = nc.alloc_sbuf_tensor("us", [C, 2, 512], F16).ap()
    vs = nc.alloc_sbuf_tensor("vs", [C, 2, 512], F16).ap()
    outsb = nc.alloc_sbuf_tensor("outsb", [C, NOBP, 8, W], F16).ap()

    pr = [nc.alloc_psum_tensor(f"pr{j}", [C, 1024], F32).ap() for j in range(4)]

    sem_x = [nc.alloc_semaphore(f"sx{i}") for i in range(NCH[0] + NCH[1])]
    sem_w = [nc.alloc_semaphore(f"sw{s}") for s in range(SPC)]
    sem_pe = nc.alloc_semaphore("pe_unit")
    sem_ev = nc.alloc_semaphore("ms_ev")
    sem_dsf = nc.alloc_semaphore("dve_self")
    sem_cmb = nc.alloc_semaphore("cmb")       # +2 per completed pair
    sem_od = [nc.alloc_semaphore(f"sod{j}") for j in range(NOBP)]

    with nc.Block() as blk:

        @blk.sync
        def _(eng):
            # output DMAs only on this HWDGE ring (8 rows per pair)
            for k in range(NPAIR):
                s, g = (2 * k) // NU, ((2 * k) % NU) // 2
                eng.wait_ge(sem_cmb, 2 * k + 2)
                eng.dma_start(
                    out=y_d[s : s + 1, :, 8 * g : 8 * g + 8, :],
                    in_=outsb[:, k % NOBP : k % NOBP + 1, :, :],
                ).then_inc(sem_od[k % NOBP], 16)

        @blk.scalar
        def _(eng):
            # ALL input DMAs ride the ACT HWDGE ring (FIFO) in consumption
            # order: conv-gating pieces up front, long-tail chunks
            # interleaved into the evict loop so evict 0 isn't delayed by
            # descriptor-issue time
            def xchunk(s, ci):
                t0, t1 = TCHS[s][ci], TCHS[s][ci + 1]
                eng.dma_start(
                    out=xs[:, s : s + 1, t0:t1, :, :],
                    in_=xt_d[s : s + 1, :, t0:t1, :, :],
                ).then_inc(sem_x[NCH[0] * s + ci], 16)

            def wdma(s):
                eng.dma_start(
                    out=wqs[:, s : s + 1, :],
                    in_=wq_d[:, s * 12 * C : (s + 1) * 12 * C],
                ).then_inc(sem_w[s], 16)

            # ring (FIFO) order = consumption order; w1 rides mid-stream
            wdma(0)
            xchunk(0, 0)
            xchunk(0, 1)
            xchunk(0, 2)
            late = {0: (0, 3), 2: (0, 4), 6: (0, 5), 8: ("w", 1),
                    10: (1, 0), 18: (1, 1), 30: (1, 2)}

            # evict all four M points of each unit region to fp16 SBUF
            for uu in range(SPC * NU):
                eng.wait_ge(sem_pe, uu + 1)
                if uu >= NMS:
                    eng.wait_ge(sem_cmb, 2 * ((uu - NMS) // 2 + 1))
                eng.copy(ms[:, uu % NMS : uu % NMS + 1, :],
                         pr[uu % 4]).then_inc(sem_ev, 1)
                if uu in late:
                    a, b = late[uu]
                    wdma(b) if a == "w" else xchunk(a, b)

        @blk.vector
        def _(eng):
            for k in range(NPAIR):
                eng.wait_ge(sem_ev, 2 * k + 2)   # both units evicted
                if k >= NOBP:
                    eng.wait_ge(sem_od[k % NOBP], 16 * (k // NOBP))
                sl = (2 * k) % NMS               # slots sl, sl+1
                mpair = ms[:, sl : sl + 2, :].rearrange(
                    "c s (p n) -> c s p n", p=P4)
                m0, m1, m2, m3 = (mpair[:, :, j : j + 1, :] for j in range(4))
                ob = k % NOBP
                eng.tensor_tensor(us[:, k % 2 : k % 2 + 1, :],
                                  m1, m2, ADD).then_inc(sem_dsf, 1)
                eng.tensor_tensor(vs[:, k % 2 : k % 2 + 1, :],
                                  m1, m2, SUB).then_inc(sem_dsf, 1)
                eng.wait_ge(sem_dsf, 2 * k + 1)   # u drained
                eng.tensor_tensor(outsb[:, ob : ob + 1, 0:8:2, :],
                                  us[:, k % 2 : k % 2 + 1, :],
                                  m0, ADD)
                eng.wait_ge(sem_dsf, 2 * k + 2)   # v drained
                eng.tensor_tensor(outsb[:, ob : ob + 1, 1:8:2, :],
                                  vs[:, k % 2 : k % 2 + 1, :],
                                  m3, SUB).then_inc(sem_cmb, 2)

        @blk.tensor
        def _(eng):
            for _ in range(NWARM):
                eng.matmul(out=pr[3][:, 0:512], lhsT=wqs[:, 0:1, 0:C],
                           rhs=xs[:, 0:1, 0:1, 0:4, 0:W], start=True, stop=True)

            for s in range(SPC):
                eng.wait_ge(sem_w[s], 16)
                for u in range(NU):
                    uu = s * NU + u
                    if u == 0 or _chunk_of_unit(s, u) != _chunk_of_unit(s, u - 1):
                        eng.wait_ge(sem_x[NCH[0] * s + _chunk_of_unit(s, u)], 16)
                    if uu >= 4:
                        # region recycles after the 1-hop ACT evict
                        eng.wait_ge(sem_ev, uu - 3)
                    t0 = TPU * u
                    for p in range(P4):
                        for kw in range(KS):
                            ws = p * KS + kw
                            inst = eng.matmul(
                                out=pr[uu % 4][:, p * 256 : (p + 1) * 256],
                                lhsT=wqs[:, s : s + 1, ws * C : (ws + 1) * C],
                                rhs=xs[:, s : s + 1, t0 : t0 + TPU,
                                       p : p + 1, kw : kw + W],
                                start=(kw == 0),
                                stop=(kw == KS - 1),
                            )
                    inst.then_inc(sem_pe, 1)

    return nc


def _host_prep(x, w, weight, mod_w, mod_b):
    import ml_dtypes
    bf16 = ml_dtypes.bfloat16
    f = np.float32
    x = np.asarray(x, f)
    w = np.asarray(w, f)
    weight = np.asarray(weight, f)
    mod_w = np.asarray(mod_w, f)
    mod_b = np.asarray(mod_b, f)

    # modulation + demodulation folded into per-sample weights
    s = (w @ mod_w.T + mod_b) + 1.0
    wgt = weight[None] * s[:, None, :, None, None]            # [B, O, I, 3, 3]
    d = 1.0 / np.sqrt((wgt * wgt).sum(axis=(2, 3, 4)) + 1e-8)
    g = wgt * d[:, :, None, None, None]

    # Winograd G-transform along kh: g~[p, kw]; layout wq[i, s*12C+(p*3+kw)*C+o]
    gt = np.stack([g[:, :, :, 0, :],
                   (g[:, :, :, 0, :] + g[:, :, :, 1, :] + g[:, :, :, 2, :]) * 0.5,
                   (g[:, :, :, 0, :] - g[:, :, :, 1, :] + g[:, :, :, 2, :]) * 0.5,
                   g[:, :, :, 2, :]], axis=3)                 # [B, O, I, 4, 3]
    wq = np.ascontiguousarray(gt.transpose(2, 0, 3, 4, 1)).reshape(C, B, 12 * C)

    # input transform along H (padded rows/cols), tile-major [B, I, 64, 4, 130]
    xp = np.zeros((B, C, H + 2, W + 2), f)
    xp[:, :, 1 : H + 1, 1 : W + 1] = x
    e, o1, o2, o3 = (xp[:, :, 0:128:2, :], xp[:, :, 1:129:2, :],
                     xp[:, :, 2:130:2, :], xp[:, :, 3:131:2, :])
    xt = np.stack([e - o2, o1 + o2, o2 - o1, o1 - o3], axis=3).astype(bf16)

    in_maps = []
    for core in range(NCORES):
        s0 = SPC * core
        in_maps.append({
            "xt": np.ascontiguousarray(xt[s0 : s0 + SPC]),
            "wq": np.ascontiguousarray(
                wq[:, s0 : s0 + SPC].reshape(C, SPC * 12 * C)).astype(bf16),
        })
    return in_maps


def _gather(res):
    return np.concatenate(
        [res.results[i]["y"].astype(np.float32) for i in range(NCORES)], axis=0)


_cached = {}


def kernel(x, w, weight, mod_w, mod_b):
    if "nc" not in _cached:
        _cached["nc"] = build_program()
    nc = _cached["nc"]
    in_maps = _host_prep(x, w, weight, mod_w, mod_b)
    res = run_bass_kernel_spmd(nc, in_maps, list(range(NCORES)))
    return _gather(res)


if __name__ == "__main__":
    from concourse.bass_utils import compile_bass_kernel
    import tempfile

    nc = build_program()
    d = tempfile.mkdtemp()
    neff = compile_bass_kernel(nc, d)
    print("compiled OK:", neff)
